# revision 23
# baseline (speedup 1.0000x reference)
"""AIMNet2 interaction module on 8 TRN2 NeuronCores.

Strategy
--------
Algebraic restructure: the nn.Linear commutes with the segment-sum, so we
accumulate A[n, ch, f] = sum_{p: idx_i[p]=n} c_ch[p] * E[idx_j[p], f] with
c = f_ij * [1, ux, uy, uz] (4 channels), then apply W on the [N,3,F] result
plus a count_n * b correction, then the norm.  This cuts matmul work 16x and
avoids materializing [P,3,F].

Sharding: pairs sorted by idx_i on host; each core owns a contiguous range of
2500 target atoms and all pairs whose idx_i lands in it -> zero inter-core
communication.  Atoms are greedy-packed into windows (<=32 atoms AND <=512
pairs each; ~84 windows/core, adaptive, multiple of 4).

Per-core device pipeline (raw bass, manual semaphores):
  gpsimd : dma_gather calls (1024 rows x 256B bf16) of neighbor embeddings,
           round-robin over 4 SWDGE queues so desc-gen runs on all 4 Q7 cpu
           pairs concurrently (~4x the single-queue desc-gen rate); this
           stream sets the kernel runtime (~2.2us per 1024 rows).
  PE     : per 128-pair chunk: bank[f, (w%4)*128+(ch,a)] += a_j^T @ wone
           (4 windows share a 512-col psum bank); per 4-window group the
           W-transform psum2[g, (w,c,a)] = wt.T @ vec + b x counts.
  ACT    : one full-bank evac per group (f32->bf16, rad+vec together),
           Square, Sqrt.
  DVE    : per-group sum-of-squares adds only (wone comes precomputed from
           the host, streamed via DMA into a 4-group SBUF ring).
  sync   : input DMAs (gidx in 4 pieces, wone in per-group pieces) + per-group
           streamed output DMAs.
"""
import sys
import numpy as np
import ml_dtypes

sys.path.insert(0, "/opt/trn_rl_repo")

import concourse.bass as bass
import concourse.bacc as bacc
import concourse.mybir as mybir
from concourse.bass_utils import run_bass_kernel_spmd
from concourse.library_config import mlp

# ---------------- problem constants (hardcoded per spec) ----------------
N_ATOMS = 20000
F = 128
N_CORES = 8
ATOMS_PER_CORE = 2500          # 8 * 2500 = 20000
WIN = 16                       # max atoms per window
N_WIN = 168                    # windows per core (greedy-packed; adaptive)
N_LOC = N_WIN * WIN            # padded local atom slots
K_CH = 2                       # chunks (x128 pairs) per window -> 256 slots
SLOTS_PER_WIN = K_CH * 128
N_SLOT = N_WIN * SLOTS_PER_WIN
N_CHUNK = N_SLOT // 128
GCH = 1024                     # rows per dma_gather call
N_GATHER = N_SLOT // GCH
GRP = 8                        # windows per group (psum bank / phase-2 unit)
N_GRP = N_WIN // GRP
WPG = 3                        # groups per wone upload piece
EPS = 1e-12

N_Q = 4                        # SWDGE queues (desc-gen cpu pairs), round-robin
QDEPTH = 4                     # outstanding gathers per queue (16 total)

bf16 = mybir.dt.bfloat16
f32 = mybir.dt.float32
i16 = mybir.dt.int16

_cache = {}


# queue 0's gather holds the GpSimd engine for its whole desc-gen, while
# queues 1-3 dispatch in ~60ns and desc-gen in background.  Issue the
# background queues FIRST each round so their desc-gen overlaps the queue-0
# hold instead of trailing it (cuts a full desc-gen off the stream tail).
_PERM = (1, 2, 3, 0)


def _gq(s):
    return _PERM[s % N_Q]


def _qcnt(s, q):
    """#gathers with index <= s on queue q."""
    r = _PERM.index(q)
    if s < r:
        return 0
    return (s - r) // N_Q + 1


def _build_graph():
    nc = bacc.Bacc("TRN2", debug=False, num_swdge_queues=N_Q)
    dp = nc.declare_dram_parameter
    table = dp("table", [N_ATOMS, F], bf16, isOutput=False)
    gidx = dp("gidx", [128, N_SLOT // 16], i16, isOutput=False)
    wone = dp("wone", [128, N_WIN, K_CH, 4 * WIN], bf16, isOutput=False)
    wt = dp("wt", [F, F], bf16, isOutput=False)          # W transposed
    bvec = dp("bvec", [1, F], bf16, isOutput=False)
    cnt3 = dp("cnt3", [1, N_WIN * 3 * WIN], bf16, isOutput=False)
    out_v = dp("out_v", [128, N_LOC], bf16, isOutput=True)  # vector norms
    out_r = dp("out_r", [128, N_LOC], bf16, isOutput=True)  # radial

    import contextlib
    with contextlib.ExitStack() as ctx:
        E = ctx.enter_context
        block = E(nc.Block())
        gath = E(nc.sbuf_tensor("gath", [128, N_CHUNK, F], bf16))
        gidx_sb = E(nc.sbuf_tensor("gidx_sb", [128, N_SLOT // 16], i16))
        wone_sb = E(nc.sbuf_tensor("wone_sb", [128, N_GRP, GRP, K_CH, 4 * WIN], bf16))
        wt_sb = E(nc.sbuf_tensor("wt_sb", [F, F], bf16))
        bvec_sb = E(nc.sbuf_tensor("bvec_sb", [1, F], bf16))
        cnt3_sb = E(nc.sbuf_tensor("cnt3_sb", [1, N_WIN * 3 * WIN], bf16))
        # per-group evac targets (contiguous so out-DMAs use big descriptors)
        rad_sb = E(nc.sbuf_tensor("rad_sb", [128, N_LOC], bf16))
        vec_sb = E(nc.sbuf_tensor("vec_sb", [128, N_WIN * 3 * WIN], bf16))
        vnorm_sb = E(nc.sbuf_tensor("vnorm_sb", [128, N_LOC], bf16))
        sq_sb = E(nc.sbuf_tensor("sq_sb", [128, 2, GRP * 3 * WIN], f32))
        vsq_sb = E(nc.sbuf_tensor("vsq_sb", [128, 2, GRP * WIN], f32))
        eps_sb = E(nc.sbuf_tensor("eps_sb", [128, 1], f32))
        banks = [E(nc.psum_tensor(f"bank{i}", [128, GRP * 4 * WIN], f32))
                 for i in range(4)]
        psum2 = [E(nc.psum_tensor(f"ps2_{i}", [128, GRP * 3 * WIN], f32))
                 for i in range(2)]

        io = E(nc.semaphore("io"))
        # per-piece sems: concurrent DMAs complete out of order, so a single
        # counting semaphore cannot express "pieces 0..k landed"
        gisems = [E(nc.semaphore(f"gisem{k}")) for k in range(N_Q)]
        gsems = [E(nc.semaphore(f"gsem{q}")) for q in range(N_Q)]
        n_wp = -(-N_GRP // WPG)
        wsems = [E(nc.semaphore(f"wsem{k}")) for k in range(n_wp)]
        pe_win = E(nc.semaphore("pe_win"))
        evac = E(nc.semaphore("evac"))
        pe2 = E(nc.semaphore("pe2"))
        sqs = E(nc.semaphore("sqs"))
        vsqs = E(nc.semaphore("vsqs"))
        vns = E(nc.semaphore("vns"))
        outs = E(nc.semaphore("outs"))

        @block.gpsimd
        def _(g: bass.BassGpSimd):
            g.load_library(mlp)
            nreg = g.to_reg(GCH)   # hoisted: one MOVE instead of one per call
            cpg = GCH // 128   # sbuf chunks per gather
            ipg = GCH // 16    # idx cols per gather
            per_piece = -(-N_GATHER // N_Q)  # gidx arrives in N_Q pieces
            seen_piece = -1
            for s in range(N_GATHER):
                piece = s // per_piece
                if piece > seen_piece:
                    g.wait_ge(gisems[piece], 16)
                    seen_piece = piece
                q = _gq(s)
                k = s // N_Q
                if k >= QDEPTH:
                    g.wait_ge(gsems[q], 16 * (k - QDEPTH + 1))
                g.dma_gather(
                    gath[:, s * cpg:(s + 1) * cpg, :],
                    table[:],
                    gidx_sb[:, s * ipg:(s + 1) * ipg],
                    GCH, nreg, F,
                    queue_num=q,
                ).then_inc(gsems[q], 16)
            # hold off the end-of-block ring drain until every gather's DMA
            # has landed: the drain throttles in-flight SWDGE transfers ~12x
            for q in range(N_Q):
                g.wait_ge(gsems[q], 16 * _qcnt(N_GATHER - 1, q))

        @block.tensor
        def _(t: bass.BassTensorEngine):
            t.wait_ge(io, 16 * 3)   # wt, bvec, cnt3

            def phase2(g):
                t.wait_ge(evac, g + 1)
                if g >= 2:
                    t.wait_ge(sqs, g - 1)     # psum2 slot reuse
                t.matmul(
                    out=psum2[g % 2][:],
                    lhsT=wt_sb[:],
                    rhs=vec_sb[:, g * GRP * 3 * WIN:(g + 1) * GRP * 3 * WIN],
                    start=True, stop=False,
                )
                t.matmul(
                    out=psum2[g % 2][:],
                    lhsT=bvec_sb[:],
                    rhs=cnt3_sb[:, g * GRP * 3 * WIN:(g + 1) * GRP * 3 * WIN],
                    start=False, stop=True,
                ).then_inc(pe2, 1)

            waited = [0] * N_Q
            for w in range(N_WIN):
                gi = w // GRP
                bank = banks[gi % 4]
                if w % GRP == 0:
                    t.wait_ge(wsems[gi // WPG], 16)
                    if gi >= 4:
                        t.wait_ge(evac, gi - 3)   # psum bank reuse
                last_g = (K_CH * w + K_CH - 1) // (GCH // 128)
                for q in range(N_Q):
                    cnt = _qcnt(last_g, q)
                    if cnt > waited[q]:
                        t.wait_ge(gsems[q], 16 * cnt)
                        waited[q] = cnt
                c0 = (w % GRP) * 4 * WIN
                for k in range(K_CH):
                    mm = t.matmul(
                        out=bank[:, c0:c0 + 4 * WIN],
                        lhsT=gath[:, K_CH * w + k, :],
                        rhs=wone_sb[:, gi, w % GRP, k, :],
                        start=(k == 0),
                        stop=(k == K_CH - 1),
                    )
                    if k == K_CH - 1:
                        mm.then_inc(pe_win, 1)
                # phase 2 lags one group so PE never idles on the evac chain
                if w % GRP == GRP - 1 and gi >= 1:
                    phase2(gi - 1)
            phase2(N_GRP - 1)

        @block.scalar
        def _(a: bass.BassEngine):
            Copy = mybir.ActivationFunctionType.Copy
            for gi in range(N_GRP):
                a.wait_ge(pe_win, GRP * (gi + 1))
                bk = banks[gi % 4][:].rearrange("p (w c) -> p w c", w=GRP)
                a.activation(
                    out=rad_sb[:, gi * GRP * WIN:(gi + 1) * GRP * WIN],
                    in_=bk[:, :, 0:WIN], func=Copy)
                a.activation(
                    out=vec_sb[:, gi * GRP * 3 * WIN:(gi + 1) * GRP * 3 * WIN],
                    in_=bk[:, :, WIN:4 * WIN], func=Copy).then_inc(evac, 1)
                a.wait_ge(pe2, gi + 1)
                if gi >= 2:
                    a.wait_ge(vsqs, gi - 1)      # sq slot reuse
                a.activation(out=sq_sb[:, gi % 2], in_=psum2[gi % 2][:],
                             func=mybir.ActivationFunctionType.Square,
                             ).then_inc(sqs, 1)
                a.wait_ge(vsqs, gi + 1)
                a.activation(out=vnorm_sb[:, gi * GRP * WIN:(gi + 1) * GRP * WIN],
                             in_=vsq_sb[:, gi % 2],
                             func=mybir.ActivationFunctionType.Sqrt,
                             bias=eps_sb[:, 0:1]).then_inc(vns, 1)

        @block.vector
        def _(v: bass.BassVectorEngine):
            v.memset(eps_sb[:], EPS)
            for gi in range(N_GRP):
                v.wait_ge(sqs, gi + 1)
                if gi >= 2:
                    v.wait_ge(vns, gi - 1)       # vsq slot reuse
                s3 = sq_sb[:, gi % 2].rearrange("p (w c a) -> p w c a",
                                                c=3, a=WIN)
                v.tensor_tensor(
                    out=vsq_sb[:, gi % 2].rearrange("p (w a) -> p w a", a=WIN),
                    in0=s3[:, :, 0, :],
                    in1=s3[:, :, 1, :],
                    op=mybir.AluOpType.add,
                )
                v.tensor_tensor(
                    out=vsq_sb[:, gi % 2].rearrange("p (w a) -> p w a", a=WIN),
                    in0=vsq_sb[:, gi % 2].rearrange("p (w a) -> p w a", a=WIN),
                    in1=s3[:, :, 2, :],
                    op=mybir.AluOpType.add,
                ).then_inc(vsqs, 1)

        @block.sync
        def _(s: bass.BassEngine):
            ipg = GCH // 16
            per_piece = -(-N_GATHER // N_Q)
            for k in range(N_Q):
                lo = k * per_piece * ipg
                hi = min((k + 1) * per_piece, N_GATHER) * ipg
                s.dma_start(gidx_sb[:, lo:hi], gidx[:, lo:hi]
                            ).then_inc(gisems[k], 16)
            s.dma_start(wt_sb[:], wt[:]).then_inc(io, 16)
            s.dma_start(bvec_sb[:], bvec[:]).then_inc(io, 16)
            s.dma_start(cnt3_sb[:], cnt3[:]).then_inc(io, 16)
            n_wp = -(-N_GRP // WPG)
            for k in range(n_wp):
                g0, g1 = k * WPG, min((k + 1) * WPG, N_GRP)
                s.dma_start(
                    wone_sb[:, g0:g1].rearrange("p g a b c -> p (g a b c)"),
                    wone[:, g0 * GRP:g1 * GRP].rearrange(
                        "p a b c -> p (a b c)"),
                ).then_inc(wsems[k], 16)
            # single big output DMAs at the end: per-group streaming floods
            # the DMA engines with 256B descriptors and starves the final
            # gather transfers
            s.wait_ge(evac, N_GRP)
            s.dma_start(out_r[:], rad_sb[:]).then_inc(outs, 16)
            s.wait_ge(vns, N_GRP)
            s.dma_start(out_v[:], vnorm_sb[:]).then_inc(outs, 16)
            s.wait_ge(outs, 32)

    nc.compile()
    return nc


def _prep_core(idx_i, idx_j, coef4, base):
    """Build per-core host arrays. idx_* already filtered+sorted by idx_i.

    Greedy variable-atom windows: consecutive local atoms are packed into a
    window until it would exceed SLOTS_PER_WIN pairs or WIN atoms."""
    a_loc = idx_i - base                       # [p] in [0, ATOMS_PER_CORE)
    counts = np.bincount(a_loc, minlength=ATOMS_PER_CORE)
    atom_win = np.zeros(ATOMS_PER_CORE, dtype=np.int64)
    atom_rank = np.zeros(ATOMS_PER_CORE, dtype=np.int64)
    w = acc = na = 0
    for atom in range(ATOMS_PER_CORE):
        c = int(counts[atom])
        if acc + c > SLOTS_PER_WIN or na == WIN:
            w += 1
            acc = na = 0
        atom_win[atom] = w
        atom_rank[atom] = na
        acc += c
        na += 1
    if w >= N_WIN:
        raise RuntimeError(f"needs {w + 1} windows > {N_WIN}")
    win = atom_win[a_loc]
    jidx = np.zeros(N_SLOT, dtype=np.int16)
    slot_rank = np.zeros(N_SLOT, dtype=np.int64)
    slot_coef = np.zeros((N_SLOT, 4), dtype=np.float32)
    cnt_w = np.bincount(win, minlength=N_WIN)
    # pairs are sorted by idx_i hence grouped by window
    starts_in = np.concatenate([[0], np.cumsum(cnt_w)[:-1]])
    for wi in range(N_WIN):
        n = cnt_w[wi]
        if n == 0:
            continue
        s0, d0 = starts_in[wi], wi * SLOTS_PER_WIN
        jidx[d0:d0 + n] = idx_j[s0:s0 + n]
        slot_rank[d0:d0 + n] = atom_rank[a_loc[s0:s0 + n]]
        slot_coef[d0:d0 + n] = coef4[s0:s0 + n]
    # gather idx wrap: per gather-call, [16, GCH//16] blocks
    gidx_h = np.tile(
        jidx.reshape(N_GATHER, GCH // 16, 16).transpose(2, 0, 1).reshape(16, -1),
        (8, 1))
    # weighted one-hot rhs, precomputed: [p, win, k, (c, a)]
    wone_flat = np.zeros((N_SLOT, 4, WIN), dtype=np.float32)
    wone_flat[np.arange(N_SLOT), :, slot_rank] = slot_coef
    wone_h = np.ascontiguousarray(
        wone_flat.reshape(N_WIN, K_CH, 128, 4 * WIN).transpose(2, 0, 1, 3)
    ).astype(ml_dtypes.bfloat16)
    # counts replicated over 3 vec channels: [w, c, a-rank]
    col_of = (atom_win * WIN + atom_rank).astype(np.int64)
    cnts_col = np.zeros(N_LOC, dtype=np.float32)
    cnts_col[col_of] = counts
    cnt3_h = np.broadcast_to(
        cnts_col.reshape(N_WIN, 1, WIN), (N_WIN, 3, WIN)).reshape(1, -1)
    return (gidx_h, wone_h,
            np.ascontiguousarray(cnt3_h).astype(ml_dtypes.bfloat16), col_of)


def _windows_needed(a_loc):
    counts = np.bincount(a_loc, minlength=ATOMS_PER_CORE)
    w = acc = na = 0
    for atom in range(ATOMS_PER_CORE):
        c = int(counts[atom])
        if acc + c > SLOTS_PER_WIN or na == WIN:
            w += 1
            acc = na = 0
        acc += c
        na += 1
    return w + 1


def _set_n_win(nw):
    g = globals()
    g["N_WIN"] = nw
    g["N_LOC"] = nw * WIN
    g["N_SLOT"] = nw * SLOTS_PER_WIN
    g["N_CHUNK"] = g["N_SLOT"] // 128
    g["N_GATHER"] = g["N_SLOT"] // GCH
    g["N_GRP"] = nw // GRP


def kernel(atomic_embedding, pairlist, f_ij_cutoff, r_ij, W, b):
    atomic_embedding = np.asarray(atomic_embedding, dtype=np.float32)
    pairlist = np.asarray(pairlist)
    f_ij = np.asarray(f_ij_cutoff, dtype=np.float32).reshape(-1)
    r_ij = np.asarray(r_ij, dtype=np.float32)
    W = np.asarray(W, dtype=np.float32)
    b = np.asarray(b, dtype=np.float32)

    u = r_ij / np.linalg.norm(r_ij, axis=1, keepdims=True)
    coef4 = np.concatenate([f_ij[:, None], f_ij[:, None] * u], axis=1)  # [P,4]

    idx_i = np.asarray(pairlist[0], dtype=np.int64)
    idx_j = np.asarray(pairlist[1], dtype=np.int64)
    order = np.argsort(idx_i, kind="stable")
    idx_i_s, idx_j_s, coef_s = idx_i[order], idx_j[order], coef4[order]

    table = atomic_embedding.astype(ml_dtypes.bfloat16)
    wt_h = np.ascontiguousarray(W.T).astype(ml_dtypes.bfloat16)
    b_h = b.reshape(1, F).astype(ml_dtypes.bfloat16)

    bounds = np.searchsorted(idx_i_s, np.arange(0, N_ATOMS + 1, ATOMS_PER_CORE))
    need = max(_windows_needed(idx_i_s[bounds[c]:bounds[c + 1]] - c * ATOMS_PER_CORE)
               for c in range(N_CORES))
    # round up: multiple of GRP (phase-2 groups) and of 2 (1024-idx gathers)
    nw = -(-max(need, 16) // GRP) * GRP
    if nw != N_WIN:
        _cache.pop("nc", None)
    _set_n_win(nw)
    in_maps = []
    colmaps = []
    for c in range(N_CORES):
        lo, hi = bounds[c], bounds[c + 1]
        gidx_h, wone_h, cnt3_h, col_of = _prep_core(
            idx_i_s[lo:hi], idx_j_s[lo:hi], coef_s[lo:hi], c * ATOMS_PER_CORE)
        in_maps.append({
            "table": table, "gidx": gidx_h,
            "wone": wone_h.reshape(128, N_WIN, K_CH, 4 * WIN),
            "wt": wt_h, "bvec": b_h, "cnt3": cnt3_h,
        })
        colmaps.append(col_of)

    if "nc" not in _cache:
        _cache["nc"] = _build_graph()
    res = run_bass_kernel_spmd(_cache["nc"], in_maps, core_ids=list(range(N_CORES)))

    out_full = np.empty((N_ATOMS, 2 * F), dtype=np.float32)
    for c in range(N_CORES):
        ov = np.asarray(res.results[c]["out_v"]).astype(np.float32)
        orad = np.asarray(res.results[c]["out_r"]).astype(np.float32)
        n = ATOMS_PER_CORE
        out_full[c * n:(c + 1) * n, 0:F] = ov[:, colmaps[c]].T
        out_full[c * n:(c + 1) * n, F:] = orad[:, colmaps[c]].T
    return out_full
